# revision 58
# baseline (speedup 1.0000x reference)
"""Multi-head self-attention (B=2, T=2048, C=1024, H=16, D=64) on 8 trn2 cores.

Sharding: data-parallel over (batch, T-chunk): core c = 4*b + j handles batch b,
tokens [j*512, (j+1)*512). Each core computes the full qkv projection for its
token chunk, AllGathers K and V within its batch group of 4 cores, runs
attention for all 16 heads over its 512 query rows, and applies the output
projection locally. No all-reduce needed; outputs are disjoint row slices.

Device layout notes:
- Everything transposed: xT [C, Tc], qT [C, Tc], scoresT [k, q] so that the
  softmax contraction (over k) is the matmul partition axis for attn@V.
- Softmax skips max-subtraction (scores/8 ~ N(0, 0.33); exp can't overflow).
- exp denominator comes free from a 65th all-ones column in the V tiles.
- fp32r matmuls (full PE rate at N>=256, ~tf32+ precision).
"""
import numpy as np
import concourse.bacc as bacc
import concourse.mybir as mybir
import concourse.tile as tile
from concourse.bass_utils import run_bass_kernel_spmd
from concourse.bass_interp import get_hw_module

F32 = mybir.dt.float32
F32R = mybir.dt.float32r
AF = mybir.ActivationFunctionType
MULT = mybir.AluOpType.mult

N_CORES = 8
GROUPS = [[0, 1, 2, 3], [4, 5, 6, 7]]
B, T, C, H, D = 2, 2048, 1024, 16, 64
TC = T // 4            # tokens per core: 512
KC = C // 128          # 8 contraction chunks
HPC = 2                # heads per 128-channel chunk
CCN = H // HPC         # 8 channel chunks
VW = D + 1             # v tile width with ones column
SCALE = 1.0 / np.sqrt(D)

_nc_cache = None


def _build(skip_collectives=False):
    nc = bacc.Bacc("TRN2", target_bir_lowering=False, debug=False,
                   num_devices=N_CORES)

    xT_e = nc.dram_tensor("xT", [C, TC], F32, kind="ExternalInput")
    wqkvT_e = nc.dram_tensor("wqkvT", [C, 3 * C], F32, kind="ExternalInput")
    woutT_e = nc.dram_tensor("woutT", [C, C], F32, kind="ExternalInput")
    bqk_e = nc.dram_tensor("bqk", [128, 16], F32, kind="ExternalInput")
    bv_e = nc.dram_tensor("bv", [1, C], F32, kind="ExternalInput")
    bout_e = nc.dram_tensor("bout", [128, 8], F32, kind="ExternalInput")
    onesr_e = nc.dram_tensor("onesr", [1, 128], F32, kind="ExternalInput")
    ones16_e = nc.dram_tensor("ones16", [128, 16], F32, kind="ExternalInput")
    yT_e = nc.dram_tensor("yT", [C, TC], F32, kind="ExternalOutput")

    with tile.TileContext(nc) as tc:
        with (
            tc.tile_pool(name="const", bufs=1) as constp,
            tc.tile_pool(name="xq", bufs=1) as xqp,
            tc.tile_pool(name="wstream", bufs=10) as wp,
            tc.tile_pool(name="stage", bufs=3) as stp,
            tc.tile_pool(name="ktile", bufs=2) as kp,
            tc.tile_pool(name="vtile", bufs=8) as vp,
            tc.tile_pool(name="exp", bufs=6) as ep,
            tc.tile_pool(name="norm", bufs=2) as np_,
            tc.tile_pool(name="ps", bufs=1, space="PSUM") as ps,
            tc.tile_pool(name="dram", bufs=1, space="DRAM") as dram,
            nc.allow_low_precision(reason="fp32r matmul pipeline"),
        ):
            # ---------------- constants / inputs -----------------
            bqk_t = constp.tile([128, 16], F32, name="bqk_t")
            nc.sync.dma_start(bqk_t[:], bqk_e[:, :])
            bout_t = constp.tile([128, 8], F32, name="bout_t")
            nc.sync.dma_start(bout_t[:], bout_e[:, :])
            bv_t = constp.tile([1, C], F32R, name="bv_t")
            nc.sync.dma_start(bv_t[:], bv_e[:, :].bitcast(F32R))
            onesr_t = constp.tile([1, 128], F32R, name="onesr_t")
            nc.sync.dma_start(onesr_t[:], onesr_e[:, :].bitcast(F32R))
            ones16_t = constp.tile([128, 16], F32, name="ones16_t")
            nc.sync.dma_start(ones16_t[:], ones16_e[:, :])

            # interleave x chunks with K-weight chunks so the first K matmul
            # can start after ~2 transfers
            xT_t = xqp.tile([128, KC, TC], F32R, name="xT_t", tag="xby",
                            bufs=1)
            wK1, wQ0 = [], []
            for kc in range(KC):
                nc.sync.dma_start(xT_t[:, kc, :],
                                  xT_e[kc * 128:(kc + 1) * 128, :].bitcast(F32R))
                wt = wp.tile([128, 512], F32R, name=f"wK1_{kc}", tag="w5",
                             bufs=18)
                nc.sync.dma_start(
                    wt[:], wqkvT_e[kc * 128:(kc + 1) * 128,
                                   C:C + 512].bitcast(F32R))
                wK1.append(wt)
                wt2 = wp.tile([128, 512], F32R, name=f"wQ0_{kc}", tag="w5",
                              bufs=18)
                nc.sync.dma_start(
                    wt2[:], wqkvT_e[kc * 128:(kc + 1) * 128,
                                    0:512].bitcast(F32R))
                wQ0.append(wt2)

            # internal DRAM; K gathered per-fc so comm starts ASAP
            k_bounce = [dram.tile([C // 2, TC], F32, name=f"k_bounce{i}")
                        for i in range(2)]
            k_agA = dram.tile([8 * 512, TC], F32, name="k_agA")
            kag4 = k_agA.rearrange("(f j p) t -> f p j t", j=4, p=128)
            v_bounce = [dram.tile([TC, 8 * VW], F32, name=f"v_bounce{i}")
                        for i in range(2)]
            v_ag = [dram.tile([T, 8 * VW], F32, name=f"v_ag{i}")
                    for i in range(2)]
            vb3 = [vb.rearrange("t (h u) -> t h u", u=VW) for vb in v_bounce]
            vag3 = [va.rearrange("t (h u) -> t h u", u=VW) for va in v_ag]

            qkv_ps_tags = ["o", "r"]

            def ag(in_t, out_t):
                nc.gpsimd.collective_compute(
                    "AllGather", mybir.AluOpType.bypass, replica_groups=GROUPS,
                    ins=[in_t.opt()], outs=[out_t.opt()])

            def load_w_half(col0, label, eng=None):
                """8 tiles [128, 512]: one per contraction chunk kc."""
                eng = eng or nc.sync
                tiles = []
                for kc in range(KC):
                    wt = wp.tile([128, 512], F32R, name=f"w{label}_{kc}",
                                 tag="w5", bufs=18)
                    eng.dma_start(
                        wt[:],
                        wqkvT_e[kc * 128:(kc + 1) * 128,
                                col0:col0 + 512].bitcast(F32R))
                    tiles.append(wt)
                return tiles

            def proj_fc(wtiles, fcl, fc, out_ap, bias_col, tag):
                """One 128-feature chunk of the q/k projection: out[f, t].
                fcl: 128-col offset within the weight tiles; fc: bias index."""
                ps_p = ps.tile([128, 512], F32, name=f"psp{fc}", tag=tag)
                for kc in range(KC):
                    nc.tensor.matmul(ps_p[:],
                                     wtiles[kc][:, fcl * 128:(fcl + 1) * 128],
                                     xT_t[:, kc, :],
                                     start=(kc == 0), stop=(kc == KC - 1))
                nc.vector.tensor_scalar_add(out_ap, ps_p[:], bias_col)

            def k_fc(wtiles, fc2, fc, tag):
                half = fc // 4
                k_st = stp.tile([128, TC], F32, name=f"kst{fc}", tag="st")
                proj_fc(wtiles, fc2, fc, k_st[:], bqk_t[:, 8 + fc:9 + fc], tag)
                nc.sync.dma_start(
                    k_bounce[half][fc2 * 128:(fc2 + 1) * 128, :], k_st[:])
                if not skip_collectives:
                    ag(k_bounce[half][fc2 * 128:(fc2 + 1) * 128, :],
                       k_agA[fc * 512:(fc + 1) * 512, :])

            def v_half(wtiles, vfc, tags, eng=None):
                eng = eng or nc.sync
                # tb-outer so one psum slot suffices when tags is short
                for tb in range(4):
                    pv = ps.tile([128, 512], F32, name=f"psv{vfc}_{tb}",
                                 tag=tags[tb % len(tags)])
                    for kc in range(KC):
                        nc.tensor.matmul(pv[:],
                                         xT_t[:, kc, tb * 128:(tb + 1) * 128],
                                         wtiles[kc][:],
                                         start=(kc == 0), stop=False)
                    nc.tensor.matmul(pv[:], onesr_t[0:1, :],
                                     bv_t[0:1, vfc * 512:(vfc + 1) * 512],
                                     start=False, stop=True)
                    v_st = stp.tile([128, 512], F32, name=f"vst{vfc}_{tb}",
                                    tag="st")
                    nc.vector.tensor_copy(v_st[:], pv[:])
                    eng.dma_start(
                        vb3[vfc][tb * 128:(tb + 1) * 128, :, 0:D],
                        v_st[:].rearrange("p (h u) -> p h u", u=D))
                    eng.dma_start(
                        vb3[vfc][tb * 128:(tb + 1) * 128, :, D:VW],
                        ones16_t[:, vfc * 8:(vfc + 1) * 8]
                        .rearrange("p (h u) -> p h u", u=1))
                if not skip_collectives:
                    ag(v_bounce[vfc], v_ag[vfc])

            qT_t = xqp.tile([128, CCN, TC], F32R, name="qT_t")
            ao_t = xqp.tile([128, CCN, TC], F32R, name="ao_t")
            wO = []
            y_part = None
            y_ctx = {}

            def attn_cc(cc):
                half, ccl = divmod(cc, 4)
                kt = kp.tile([128, 4, TC], F32R, name=f"kt{cc}", tag="kt",
                             bufs=2)
                vt = vp.tile([128, 16, 2 * VW], F32R, name=f"vt{cc}", tag="va",
                             bufs=2)
                if cc == 0:
                    # split the first v load per source rank so the first
                    # attnV groups start as soon as jj=0 lands
                    nc.sync.dma_start(kt[:], kag4[cc].bitcast(F32R))
                    for jj in range(4):
                        nc.sync.dma_start(
                            vt[:, 4 * jj:4 * jj + 4, :],
                            vag3[half][jj * TC:(jj + 1) * TC,
                                       2 * ccl:2 * ccl + 2, :]
                            .rearrange("(k p) h u -> p k (h u)", p=128)
                            .bitcast(F32R))
                else:
                    nc.sync.dma_start(kt[:], kag4[cc].bitcast(F32R))
                    nc.sync.dma_start(
                        vt[:],
                        vag3[half][:, 2 * ccl:2 * ccl + 2, :]
                        .rearrange("(k p) h u -> p k (h u)", p=128)
                        .bitcast(F32R))
                for beta in range(2):
                    h = 2 * cc + beta
                    lo, hi = beta * 64, beta * 64 + 64
                    ps_o = ps.tile([128, 512], F32, name=f"pso{cc}{beta}",
                                   tag="o", bufs=1)
                    for g in range(8):
                        ps_s = ps.tile([128, 2, 512], F32,
                                       name=f"pss{cc}{beta}{g}", tag="s", bufs=2)
                        for i in range(2):
                            kb = g * 2 + i
                            jj, tbl = divmod(kb, 4)
                            nc.tensor.matmul(
                                ps_s[:, i, :],
                                kt[lo:hi, jj, tbl * 128:(tbl + 1) * 128],
                                qT_t[lo:hi, cc, :], start=True, stop=True,
                                tile_position=(lo, 0))
                        ex = ep.tile([128, 2, 512], F32R,
                                     name=f"ex{cc}{beta}{g}", tag="ex")
                        nc.scalar.activation(ex[:], ps_s[:], AF.Exp, scale=SCALE)
                        for i in range(2):
                            kb = g * 2 + i
                            nc.tensor.matmul(
                                ps_o[0:VW, :],
                                vt[:, kb, beta * VW:(beta + 1) * VW],
                                ex[:, i, :],
                                start=(kb == 0), stop=(kb == 15))
                    # normalize: rows 0:64 /= row 64 (broadcast via idle GpSimd)
                    r_t = np_.tile([1, 512], F32, name=f"r{cc}{beta}", tag="r")
                    nc.vector.reciprocal(r_t[:], ps_o[D:VW, :])
                    rr = np_.tile([D, 512], F32, name=f"rr{cc}{beta}", tag="rr")
                    nc.gpsimd.partition_broadcast(rr[:], r_t[:])
                    nc.vector.tensor_tensor(ao_t[lo:hi, cc, :],
                                            ps_o[0:D, :], rr[:], op=MULT)

                if cc == 1:
                    # output projection weights; loads overlap attention
                    for c2 in range(CCN):
                        wt = wp.tile([128, 1024], F32R, name=f"wO_{c2}",
                                     tag="w10", bufs=9)
                        nc.sync.dma_start(
                            wt[:],
                            woutT_e[c2 * 128:(c2 + 1) * 128, :].bitcast(F32R))
                        wO.append(wt)
                    y_ctx["y_part"] = xqp.tile([128, 8, TC], F32,
                                               name="y_part", tag="xby", bufs=1)
                groups = {1: (0, 1), 3: (2, 3), 5: (4, 5), 6: (6,), 7: (7,)}
                if cc in groups:
                    # fold finished channel chunks into the output projection
                    # during attention PE slack
                    y_part = y_ctx["y_part"]
                    for fc in range(8):
                        ps_h = ps.tile([128, 512], F32, name=f"psh{cc}_{fc}",
                                       tag="y" if (cc < 7 or fc % 2 == 0)
                                       else "o")
                        g2 = groups[cc]
                        for c2 in g2:
                            nc.tensor.matmul(
                                ps_h[:], wO[c2][:, fc * 128:(fc + 1) * 128],
                                ao_t[:, c2, :], start=(c2 == g2[0]),
                                stop=(c2 == g2[-1]))
                        if cc == 1:
                            nc.vector.tensor_copy(y_part[:, fc, :], ps_h[:])
                        elif cc < 7:
                            nc.vector.tensor_tensor(
                                y_part[:, fc, :], ps_h[:], y_part[:, fc, :],
                                op=mybir.AluOpType.add)
                        else:
                            # in-place final add; batched DMA below
                            nc.vector.scalar_tensor_tensor(
                                y_part[:, fc, :], ps_h[:], bout_t[:, fc:fc + 1],
                                y_part[:, fc, :],
                                op0=mybir.AluOpType.add, op1=mybir.AluOpType.add)
                            if fc % 4 == 3:
                                lo4 = fc - 3
                                nc.sync.dma_start(
                                    yT_e[lo4 * 128:(lo4 + 4) * 128, :]
                                    .rearrange("(f p) t -> p f t", p=128),
                                    y_part[:, lo4:lo4 + 4, :])

            # phase order: interleaved (K-fc -> AG-fc -> Q-fc) for fc 0..3 so
            # the first head's scores can start ~25us in; V0 + AG_V0; then
            # attention cc 0..3 (high priority) with the late projections
            # (Q/K fc 4-7, V1) emitted after to fill PE slack underneath;
            # finally attention cc 4..7
            for fc in range(4):
                k_fc(wK1, fc, fc, qkv_ps_tags[fc % 2])
                proj_fc(wQ0, fc, fc, qT_t[:, fc, :], bqk_t[:, fc:fc + 1],
                        qkv_ps_tags[(fc + 1) % 2])
            wV0 = load_w_half(2 * C, "V0")
            v_half(wV0, 0, ["o", "r"])
            for cc in range(4):
                attn_cc(cc)
            wQ1 = load_w_half(512, "Q1")
            wK2 = load_w_half(C + 512, "K2")
            for fc in range(4, 8):
                k_fc(wK2, fc - 4, fc, ["r", "z"][fc % 2])
                proj_fc(wQ1, fc - 4, fc, qT_t[:, fc, :], bqk_t[:, fc:fc + 1],
                        ["z", "r"][fc % 2])
            wV1 = load_w_half(2 * C + 512, "V1")
            v_half(wV1, 1, ["r", "z"])
            for cc in range(4, CCN):
                attn_cc(cc)

    nc.compile()
    nc.m = get_hw_module(nc.m)
    return nc


def _get_nc():
    global _nc_cache
    if _nc_cache is None:
        _nc_cache = _build()
    return _nc_cache


def _in_maps(x, qkv_w, qkv_b, out_w, out_b):
    x = np.asarray(x, dtype=np.float32)
    qkv_w = np.asarray(qkv_w, dtype=np.float32)
    qkv_b = np.asarray(qkv_b, dtype=np.float32)
    out_w = np.asarray(out_w, dtype=np.float32)
    out_b = np.asarray(out_b, dtype=np.float32)

    wqkvT = np.ascontiguousarray(qkv_w.T)
    woutT = np.ascontiguousarray(out_w.T)
    bqk = np.ascontiguousarray(qkv_b[:2 * C].reshape(16, 128).T)
    bv = qkv_b[2 * C:].reshape(1, C).copy()
    bout = np.ascontiguousarray(out_b.reshape(8, 128).T)
    onesr = np.ones((1, 128), np.float32)
    ones16 = np.ones((128, 16), np.float32)

    maps = []
    for c in range(N_CORES):
        b, j = divmod(c, 4)
        xT = np.ascontiguousarray(x[b, j * TC:(j + 1) * TC, :].T)
        maps.append({
            "xT": xT, "wqkvT": wqkvT, "woutT": woutT, "bqk": bqk, "bv": bv,
            "bout": bout, "onesr": onesr, "ones16": ones16,
        })
    return maps


def _assemble(results):
    y = np.empty((B, T, C), np.float32)
    for c in range(N_CORES):
        b, j = divmod(c, 4)
        y[b, j * TC:(j + 1) * TC, :] = results[c]["yT"].T
    return y


def kernel(x, qkv_w, qkv_b, out_w, out_b):
    nc = _get_nc()
    maps = _in_maps(x, qkv_w, qkv_b, out_w, out_b)
    res = run_bass_kernel_spmd(nc, maps, list(range(N_CORES)))
    return _assemble(res.results)


def bench(x, qkv_w, qkv_b, out_w, out_b, trace=True, tmpdir=None):
    """Run with NTFF profiling; returns (y, exec_time_ns, results_obj)."""
    nc = _get_nc()
    maps = _in_maps(x, qkv_w, qkv_b, out_w, out_b)
    res = run_bass_kernel_spmd(nc, maps, list(range(N_CORES)), trace=trace,
                               tmpdir=tmpdir)
    return _assemble(res.results), res.exec_time_ns, res


# revision 59
# speedup vs baseline: 2.3407x; 2.3407x over previous
"""Multi-head self-attention (B=2, T=2048, C=1024, H=16, D=64) on 8 trn2 cores.

Sharding: data-parallel over (batch, T-chunk): core c = 4*b + j handles batch b,
tokens [j*512, (j+1)*512). Each core computes the full qkv projection for its
token chunk, AllGathers K and V within its batch group of 4 cores, runs
attention for all 16 heads over its 512 query rows, and applies the output
projection locally. No all-reduce needed; outputs are disjoint row slices.

Device layout notes:
- Everything transposed: xT [C, Tc], qT [C, Tc], scoresT [k, q] so that the
  softmax contraction (over k) is the matmul partition axis for attn@V.
- Softmax skips max-subtraction (scores/8 ~ N(0, 0.33); exp can't overflow).
- exp denominator comes free from a 65th all-ones column in the V tiles.
- fp32r matmuls (full PE rate at N>=256, ~tf32+ precision).
"""
import numpy as np
import concourse.bacc as bacc
import concourse.mybir as mybir
import concourse.tile as tile
from concourse.bass_utils import run_bass_kernel_spmd
from concourse.bass_interp import get_hw_module

F32 = mybir.dt.float32
F32R = mybir.dt.float32r
AF = mybir.ActivationFunctionType
MULT = mybir.AluOpType.mult

N_CORES = 8
GROUPS = [[0, 1, 2, 3], [4, 5, 6, 7]]
B, T, C, H, D = 2, 2048, 1024, 16, 64
TC = T // 4            # tokens per core: 512
KC = C // 128          # 8 contraction chunks
HPC = 2                # heads per 128-channel chunk
CCN = H // HPC         # 8 channel chunks
VW = D + 1             # v tile width with ones column
SCALE = 1.0 / np.sqrt(D)

_nc_cache = None


def _build(skip_collectives=False):
    nc = bacc.Bacc("TRN2", target_bir_lowering=False, debug=False,
                   num_devices=N_CORES)

    xT_e = nc.dram_tensor("xT", [C, TC], F32, kind="ExternalInput")
    wqkvT_e = nc.dram_tensor("wqkvT", [C, 3 * C], F32, kind="ExternalInput")
    woutT_e = nc.dram_tensor("woutT", [C, C], F32, kind="ExternalInput")
    bqk_e = nc.dram_tensor("bqk", [128, 16], F32, kind="ExternalInput")
    bv_e = nc.dram_tensor("bv", [1, C], F32, kind="ExternalInput")
    bout_e = nc.dram_tensor("bout", [128, 8], F32, kind="ExternalInput")
    onesr_e = nc.dram_tensor("onesr", [1, 128], F32, kind="ExternalInput")
    ones16_e = nc.dram_tensor("ones16", [128, 16], F32, kind="ExternalInput")
    yT_e = nc.dram_tensor("yT", [C, TC], F32, kind="ExternalOutput")

    with tile.TileContext(nc) as tc:
        with (
            tc.tile_pool(name="const", bufs=1) as constp,
            tc.tile_pool(name="xq", bufs=1) as xqp,
            tc.tile_pool(name="wstream", bufs=10) as wp,
            tc.tile_pool(name="stage", bufs=3) as stp,
            tc.tile_pool(name="ktile", bufs=2) as kp,
            tc.tile_pool(name="vtile", bufs=8) as vp,
            tc.tile_pool(name="exp", bufs=6) as ep,
            tc.tile_pool(name="norm", bufs=2) as np_,
            tc.tile_pool(name="ps", bufs=1, space="PSUM") as ps,
            tc.tile_pool(name="dram", bufs=1, space="DRAM") as dram,
            nc.allow_low_precision(reason="fp32r matmul pipeline"),
        ):
            # ---------------- constants / inputs -----------------
            bqk_t = constp.tile([128, 16], F32, name="bqk_t")
            nc.sync.dma_start(bqk_t[:], bqk_e[:, :])
            bout_t = constp.tile([128, 8], F32, name="bout_t")
            nc.sync.dma_start(bout_t[:], bout_e[:, :])
            bv_t = constp.tile([1, C], F32R, name="bv_t")
            nc.sync.dma_start(bv_t[:], bv_e[:, :].bitcast(F32R))
            onesr_t = constp.tile([1, 128], F32R, name="onesr_t")
            nc.sync.dma_start(onesr_t[:], onesr_e[:, :].bitcast(F32R))
            ones16_t = constp.tile([128, 16], F32, name="ones16_t")
            nc.sync.dma_start(ones16_t[:], ones16_e[:, :])

            # interleave x chunks with K-weight chunks so the first K matmul
            # can start after ~2 transfers
            xT_t = xqp.tile([128, KC, TC], F32R, name="xT_t", tag="xby",
                            bufs=1)
            wK1, wQ0 = [], []
            for kc in range(KC):
                nc.sync.dma_start(xT_t[:, kc, :],
                                  xT_e[kc * 128:(kc + 1) * 128, :].bitcast(F32R))
                wt = wp.tile([128, 512], F32R, name=f"wK1_{kc}", tag="w5",
                             bufs=18)
                nc.sync.dma_start(
                    wt[:], wqkvT_e[kc * 128:(kc + 1) * 128,
                                   C:C + 512].bitcast(F32R))
                wK1.append(wt)
                wt2 = wp.tile([128, 512], F32R, name=f"wQ0_{kc}", tag="w5",
                              bufs=18)
                nc.sync.dma_start(
                    wt2[:], wqkvT_e[kc * 128:(kc + 1) * 128,
                                    0:512].bitcast(F32R))
                wQ0.append(wt2)

            # internal DRAM; K gathered per-fc so comm starts ASAP
            k_bounce = [dram.tile([C // 2, TC], F32, name=f"k_bounce{i}")
                        for i in range(2)]
            k_ag = [dram.tile([2 * C, TC], F32, name=f"k_ag{i}")
                    for i in range(2)]
            kag3 = [ka.rearrange("(j f) t -> j f t", j=4) for ka in k_ag]
            v_bounce = [dram.tile([TC, 8 * VW], F32, name=f"v_bounce{i}")
                        for i in range(2)]
            v_ag = [dram.tile([T, 8 * VW], F32, name=f"v_ag{i}")
                    for i in range(2)]
            vb3 = [vb.rearrange("t (h u) -> t h u", u=VW) for vb in v_bounce]
            vag3 = [va.rearrange("t (h u) -> t h u", u=VW) for va in v_ag]

            qkv_ps_tags = ["o", "r"]

            def ag(in_t, out_t):
                nc.gpsimd.collective_compute(
                    "AllGather", mybir.AluOpType.bypass, replica_groups=GROUPS,
                    ins=[in_t.opt()], outs=[out_t.opt()])

            def load_w_half(col0, label, eng=None):
                """8 tiles [128, 512]: one per contraction chunk kc."""
                eng = eng or nc.sync
                tiles = []
                for kc in range(KC):
                    wt = wp.tile([128, 512], F32R, name=f"w{label}_{kc}",
                                 tag="w5", bufs=18)
                    eng.dma_start(
                        wt[:],
                        wqkvT_e[kc * 128:(kc + 1) * 128,
                                col0:col0 + 512].bitcast(F32R))
                    tiles.append(wt)
                return tiles

            def proj_fc(wtiles, fcl, fc, out_ap, bias_col, tag):
                """One 128-feature chunk of the q/k projection: out[f, t].
                fcl: 128-col offset within the weight tiles; fc: bias index."""
                ps_p = ps.tile([128, 512], F32, name=f"psp{fc}", tag=tag)
                for kc in range(KC):
                    nc.tensor.matmul(ps_p[:],
                                     wtiles[kc][:, fcl * 128:(fcl + 1) * 128],
                                     xT_t[:, kc, :],
                                     start=(kc == 0), stop=(kc == KC - 1))
                nc.vector.tensor_scalar_add(out_ap, ps_p[:], bias_col)

            def k_fc(wtiles, fc2, fc, tag):
                half = fc // 4
                k_st = stp.tile([128, TC], F32, name=f"kst{fc}", tag="st")
                proj_fc(wtiles, fc2, fc, k_st[:], bqk_t[:, 8 + fc:9 + fc], tag)
                nc.sync.dma_start(
                    k_bounce[half][fc2 * 128:(fc2 + 1) * 128, :], k_st[:])
                if fc2 == 3 and not skip_collectives:
                    ag(k_bounce[half], k_ag[half])

            def v_half(wtiles, vfc, tags, eng=None):
                eng = eng or nc.sync
                # tb-outer so one psum slot suffices when tags is short
                for tb in range(4):
                    pv = ps.tile([128, 512], F32, name=f"psv{vfc}_{tb}",
                                 tag=tags[tb % len(tags)])
                    for kc in range(KC):
                        nc.tensor.matmul(pv[:],
                                         xT_t[:, kc, tb * 128:(tb + 1) * 128],
                                         wtiles[kc][:],
                                         start=(kc == 0), stop=False)
                    nc.tensor.matmul(pv[:], onesr_t[0:1, :],
                                     bv_t[0:1, vfc * 512:(vfc + 1) * 512],
                                     start=False, stop=True)
                    v_st = stp.tile([128, 512], F32, name=f"vst{vfc}_{tb}",
                                    tag="st")
                    nc.vector.tensor_copy(v_st[:], pv[:])
                    eng.dma_start(
                        vb3[vfc][tb * 128:(tb + 1) * 128, :, 0:D],
                        v_st[:].rearrange("p (h u) -> p h u", u=D))
                    eng.dma_start(
                        vb3[vfc][tb * 128:(tb + 1) * 128, :, D:VW],
                        ones16_t[:, vfc * 8:(vfc + 1) * 8]
                        .rearrange("p (h u) -> p h u", u=1))
                if not skip_collectives:
                    ag(v_bounce[vfc], v_ag[vfc])

            qT_t = xqp.tile([128, CCN, TC], F32R, name="qT_t")
            ao_t = xqp.tile([128, CCN, TC], F32R, name="ao_t")
            wO = []
            y_part = None
            y_ctx = {}

            def attn_cc(cc):
                half, ccl = divmod(cc, 4)
                kt = kp.tile([128, 4, TC], F32R, name=f"kt{cc}", tag="kt",
                             bufs=2)
                vt = vp.tile([128, 16, 2 * VW], F32R, name=f"vt{cc}", tag="va",
                             bufs=2)
                nc.sync.dma_start(
                    kt[:], kag3[half][:, ccl * 128:(ccl + 1) * 128, :]
                    .rearrange("j f t -> f j t").bitcast(F32R))
                if cc == 0:
                    # split the first v load per source rank so the first
                    # attnV groups start as soon as jj=0 lands
                    for jj in range(4):
                        nc.sync.dma_start(
                            vt[:, 4 * jj:4 * jj + 4, :],
                            vag3[half][jj * TC:(jj + 1) * TC,
                                       2 * ccl:2 * ccl + 2, :]
                            .rearrange("(k p) h u -> p k (h u)", p=128)
                            .bitcast(F32R))
                else:
                    nc.sync.dma_start(
                        vt[:],
                        vag3[half][:, 2 * ccl:2 * ccl + 2, :]
                        .rearrange("(k p) h u -> p k (h u)", p=128)
                        .bitcast(F32R))
                for beta in range(2):
                    h = 2 * cc + beta
                    lo, hi = beta * 64, beta * 64 + 64
                    ps_o = ps.tile([128, 512], F32, name=f"pso{cc}{beta}",
                                   tag="o", bufs=1)
                    for g in range(8):
                        ps_s = ps.tile([128, 2, 512], F32,
                                       name=f"pss{cc}{beta}{g}", tag="s", bufs=2)
                        for i in range(2):
                            kb = g * 2 + i
                            jj, tbl = divmod(kb, 4)
                            nc.tensor.matmul(
                                ps_s[:, i, :],
                                kt[lo:hi, jj, tbl * 128:(tbl + 1) * 128],
                                qT_t[lo:hi, cc, :], start=True, stop=True,
                                tile_position=(lo, 0))
                        ex = ep.tile([128, 2, 512], F32R,
                                     name=f"ex{cc}{beta}{g}", tag="ex")
                        nc.scalar.activation(ex[:], ps_s[:], AF.Exp, scale=SCALE)
                        for i in range(2):
                            kb = g * 2 + i
                            nc.tensor.matmul(
                                ps_o[0:VW, :],
                                vt[:, kb, beta * VW:(beta + 1) * VW],
                                ex[:, i, :],
                                start=(kb == 0), stop=(kb == 15))
                    # normalize: rows 0:64 /= row 64 (broadcast via idle GpSimd)
                    r_t = np_.tile([1, 512], F32, name=f"r{cc}{beta}", tag="r")
                    nc.vector.reciprocal(r_t[:], ps_o[D:VW, :])
                    rr = np_.tile([D, 512], F32, name=f"rr{cc}{beta}", tag="rr")
                    nc.gpsimd.partition_broadcast(rr[:], r_t[:])
                    nc.vector.tensor_tensor(ao_t[lo:hi, cc, :],
                                            ps_o[0:D, :], rr[:], op=MULT)

                if cc == 1:
                    # output projection weights; loads overlap attention
                    for c2 in range(CCN):
                        wt = wp.tile([128, 1024], F32R, name=f"wO_{c2}",
                                     tag="w10", bufs=9)
                        nc.sync.dma_start(
                            wt[:],
                            woutT_e[c2 * 128:(c2 + 1) * 128, :].bitcast(F32R))
                        wO.append(wt)
                    y_ctx["y_part"] = xqp.tile([128, 8, TC], F32,
                                               name="y_part", tag="xby", bufs=1)
                groups = {1: (0, 1), 3: (2, 3), 5: (4, 5), 6: (6,), 7: (7,)}
                if cc in groups:
                    # fold finished channel chunks into the output projection
                    # during attention PE slack
                    y_part = y_ctx["y_part"]
                    for fc in range(8):
                        ps_h = ps.tile([128, 512], F32, name=f"psh{cc}_{fc}",
                                       tag="y" if (cc < 7 or fc % 2 == 0)
                                       else "o")
                        g2 = groups[cc]
                        for c2 in g2:
                            nc.tensor.matmul(
                                ps_h[:], wO[c2][:, fc * 128:(fc + 1) * 128],
                                ao_t[:, c2, :], start=(c2 == g2[0]),
                                stop=(c2 == g2[-1]))
                        if cc == 1:
                            nc.vector.tensor_copy(y_part[:, fc, :], ps_h[:])
                        elif cc < 7:
                            nc.vector.tensor_tensor(
                                y_part[:, fc, :], ps_h[:], y_part[:, fc, :],
                                op=mybir.AluOpType.add)
                        else:
                            # in-place final add; batched DMA below
                            nc.vector.scalar_tensor_tensor(
                                y_part[:, fc, :], ps_h[:], bout_t[:, fc:fc + 1],
                                y_part[:, fc, :],
                                op0=mybir.AluOpType.add, op1=mybir.AluOpType.add)
                            if fc % 4 == 3:
                                lo4 = fc - 3
                                nc.sync.dma_start(
                                    yT_e[lo4 * 128:(lo4 + 4) * 128, :]
                                    .rearrange("(f p) t -> p f t", p=128),
                                    y_part[:, lo4:lo4 + 4, :])

            # phase order: interleaved (K-fc -> AG-fc -> Q-fc) for fc 0..3 so
            # the first head's scores can start ~25us in; V0 + AG_V0; then
            # attention cc 0..3 (high priority) with the late projections
            # (Q/K fc 4-7, V1) emitted after to fill PE slack underneath;
            # finally attention cc 4..7
            for fc in range(4):
                k_fc(wK1, fc, fc, qkv_ps_tags[fc % 2])
                proj_fc(wQ0, fc, fc, qT_t[:, fc, :], bqk_t[:, fc:fc + 1],
                        qkv_ps_tags[(fc + 1) % 2])
            wV0 = load_w_half(2 * C, "V0")
            v_half(wV0, 0, ["o", "r"])
            for cc in range(4):
                attn_cc(cc)
            wQ1 = load_w_half(512, "Q1")
            wK2 = load_w_half(C + 512, "K2")
            for fc in range(4, 8):
                k_fc(wK2, fc - 4, fc, ["r", "z"][fc % 2])
                proj_fc(wQ1, fc - 4, fc, qT_t[:, fc, :], bqk_t[:, fc:fc + 1],
                        ["z", "r"][fc % 2])
            wV1 = load_w_half(2 * C + 512, "V1")
            v_half(wV1, 1, ["r", "z"])
            for cc in range(4, CCN):
                attn_cc(cc)

    nc.compile()
    nc.m = get_hw_module(nc.m)
    return nc


def _get_nc():
    global _nc_cache
    if _nc_cache is None:
        _nc_cache = _build()
    return _nc_cache


def _in_maps(x, qkv_w, qkv_b, out_w, out_b):
    x = np.asarray(x, dtype=np.float32)
    qkv_w = np.asarray(qkv_w, dtype=np.float32)
    qkv_b = np.asarray(qkv_b, dtype=np.float32)
    out_w = np.asarray(out_w, dtype=np.float32)
    out_b = np.asarray(out_b, dtype=np.float32)

    wqkvT = np.ascontiguousarray(qkv_w.T)
    woutT = np.ascontiguousarray(out_w.T)
    bqk = np.ascontiguousarray(qkv_b[:2 * C].reshape(16, 128).T)
    bv = qkv_b[2 * C:].reshape(1, C).copy()
    bout = np.ascontiguousarray(out_b.reshape(8, 128).T)
    onesr = np.ones((1, 128), np.float32)
    ones16 = np.ones((128, 16), np.float32)

    maps = []
    for c in range(N_CORES):
        b, j = divmod(c, 4)
        xT = np.ascontiguousarray(x[b, j * TC:(j + 1) * TC, :].T)
        maps.append({
            "xT": xT, "wqkvT": wqkvT, "woutT": woutT, "bqk": bqk, "bv": bv,
            "bout": bout, "onesr": onesr, "ones16": ones16,
        })
    return maps


def _assemble(results):
    y = np.empty((B, T, C), np.float32)
    for c in range(N_CORES):
        b, j = divmod(c, 4)
        y[b, j * TC:(j + 1) * TC, :] = results[c]["yT"].T
    return y


def kernel(x, qkv_w, qkv_b, out_w, out_b):
    nc = _get_nc()
    maps = _in_maps(x, qkv_w, qkv_b, out_w, out_b)
    res = run_bass_kernel_spmd(nc, maps, list(range(N_CORES)))
    return _assemble(res.results)


def bench(x, qkv_w, qkv_b, out_w, out_b, trace=True, tmpdir=None):
    """Run with NTFF profiling; returns (y, exec_time_ns, results_obj)."""
    nc = _get_nc()
    maps = _in_maps(x, qkv_w, qkv_b, out_w, out_b)
    res = run_bass_kernel_spmd(nc, maps, list(range(N_CORES)), trace=trace,
                               tmpdir=tmpdir)
    return _assemble(res.results), res.exec_time_ns, res
